# revision 28
# baseline (speedup 1.0000x reference)
"""Trainium2 Bass kernel for nn_GCEncoderLayer_78400333021790.

GC encoder layer: per-node MHA over T=12 steps + FFN (both with residual+LN),
then a 3-support graph convolution over the 325-node sensor graph.

Strategy (data-parallel over batch B=32 -> 4 batches per core, 8 cores):
  - token order per core: (b, n, t); activations kept feature-major
    X^T = (d=128 partitions, tokens free) so every projection is a natural
    PE matmul.  All persistent activations in bf16.
  - MHA algebra folded on CPU:  S^T = (X Wqk^T) X^T with Wqk = Wq Wk^T/sqrt(128)
    (bq=bk=0 per spec), Vt = X (Wv Wo) so the output projection disappears.
  - groups of 10 nodes (120 tokens) per attention step; block-diagonal mask
    realized as a rank-11 matmul pre-loaded into PSUM (exp underflows to 0).
  - softmax normalization: exp (ACT) -> partition_all_reduce to bf16 (GPSIMD)
    -> single bf16 tensor-tensor divide (DVE, 2x mode).
  - LayerNorm in feature-major: bf16 column sums via ones-matmuls; gamma/beta
    folded into the downstream weights (ffn_W1 / gc_kernel / bias rows), so
    LN emits the bare normalized value via one bf16 divide.
  - PSUM->SBUF evictions load-balanced across ACT / DVE / GPSIMD with a
    static cost model (GPSIMD is otherwise idle).
  - GCN: out = Z G0 + A0 (Z G1) + A1 (Z G2) + bias with dense A built on CPU;
    Z stays in (b, n, t) order and the per-t node tiles are read through
    strided APs.
"""

import os
import sys

for _p in ("/opt/trn_rl_repo", "/root/.axon_site/_ro/trn_rl_repo"):
    if os.path.isdir(_p) and _p not in sys.path:
        sys.path.insert(0, _p)

from contextlib import ExitStack

import ml_dtypes
import numpy as np

import concourse.bass as bass
import concourse.bass_isa as bass_isa
import concourse.tile as tile
from concourse import bacc, mybir

N = 325
T = 12
D = 128
H = 8
DFF = 512
NCORES = 8
B_TOT = 32
LN_EPS = 1e-3
SQRT_D = float(np.sqrt(128.0))

BF = mybir.dt.bfloat16
F32 = mybir.dt.float32
AL = mybir.AluOpType
AF = mybir.ActivationFunctionType
bf16 = ml_dtypes.bfloat16

NODE_TILES = [(0, 128), (128, 128), (256, 69)]
GROUPS = [(i * 10, 10) for i in range(32)] + [(320, 5)]
BIG = 173.0  # sqrt(~30000); exp(-BIG^2) == 0 in fp32


def _r(x):
    return np.ascontiguousarray(x)


def _bf(x):
    return _r(np.asarray(x, np.float32).astype(bf16))


def make_consts(inp):
    """CPU-side weight folding. Returns dict of extra dram inputs (shared
    across cores)."""
    Wq = np.asarray(inp["Wq"], np.float32)
    Wk = np.asarray(inp["Wk"], np.float32)
    Wv = np.asarray(inp["Wv"], np.float32)
    Wo = np.asarray(inp["Wo"], np.float32)
    bv = np.asarray(inp["bv"], np.float32)
    bo = np.asarray(inp["bo"], np.float32)
    ln1_g = np.asarray(inp["ln1_g"], np.float32)
    ln1_b = np.asarray(inp["ln1_b"], np.float32)
    ln2_g = np.asarray(inp["ln2_g"], np.float32)
    ln2_b = np.asarray(inp["ln2_b"], np.float32)

    # wqkT[:, h*D:(h+1)*D][d, e] = Wqk_h[e, d],  Wqk_h = Wq_h Wk_h^T / sqrt(D)
    wqkT = np.empty((D, H * D), np.float32)
    wvo = np.empty((D, H * D), np.float32)
    for h in range(H):
        wqk_h = (Wq[:, h, :] @ Wk[:, h, :].T) / SQRT_D  # (D, D)
        wqkT[:, h * D:(h + 1) * D] = wqk_h.T
        wvo[:, h * D:(h + 1) * D] = Wv[:, h, :] @ Wo[h]  # (D, D)
    bvo = (np.einsum("hk,hkd->d", bv, Wo) + bo).astype(np.float32)

    # block-diag 0/1 masks on the packed (s, h*gt+t) softmax layout
    def _bmask(gn):
        gt = gn * 12
        m = np.zeros((120, H * gt), np.float32)
        for h in range(H):
            for blk in range(gn):
                m[blk * 12:(blk + 1) * 12,
                  h * gt + blk * 12:h * gt + (blk + 1) * 12] = 1.0
        return m

    A0 = np.zeros((N, N), np.float32)
    A1 = np.zeros((N, N), np.float32)
    np.add.at(A0, (np.asarray(inp["sup0_rows"]), np.asarray(inp["sup0_cols"])),
              np.asarray(inp["sup0_vals"], np.float32))
    np.add.at(A1, (np.asarray(inp["sup1_rows"]), np.asarray(inp["sup1_cols"])),
              np.asarray(inp["sup1_vals"], np.float32))

    G = np.asarray(inp["gc_kernel"], np.float32)  # (3D, D), rows ordered (d, m)
    G0, G1, G2 = G[0::3], G[1::3], G[2::3]  # each (D, D)

    # LN2 fold: GCN input is zhat*g2 + b2  ->  scale G rows by g2, push the
    # b2 term into per-support constant rows added via the bias matmul.
    g012 = np.concatenate([ln2_g[:, None] * G1, ln2_g[:, None] * G2,
                           ln2_g[:, None] * G0], axis=1)
    r0 = np.asarray(inp["gc_bias"], np.float32) + ln2_b @ G0
    r1 = ln2_b @ G1
    r2 = ln2_b @ G2
    gcb3 = np.stack([np.tile(r0, 4), np.tile(r1, 4), np.tile(r2, 4)])  # (3,512)
    brows = np.stack([np.ones(N, np.float32), A0.sum(1), A1.sum(1)])  # (3, N)

    # LN1 fold: FFN input is zhat*g1 + b1ln -> scale W1 rows, push b1ln@W1
    # into the relu bias; the residual keeps zhat so the g1 scale rides the
    # final scalar_tensor_tensor and (b1ln + ffn_b2) rides a rank-1 matmul.
    W1 = np.asarray(inp["ffn_W1"], np.float32)
    w1p = ln1_g[:, None] * W1
    b1p = np.asarray(inp["ffn_b1"], np.float32) + ln1_b @ W1
    fb1r = b1p.reshape(4, 128).T  # (128, 4)
    fb2row = (np.asarray(inp["ffn_b2"], np.float32) + ln1_b)[None, :]  # (1,128)

    w2 = np.asarray(inp["ffn_W2"], np.float32)  # (DFF, D)
    w2r = w2.reshape(4, 128, D).transpose(1, 0, 2)  # (128, 4, D)

    # centering matrix for LN: xc = (I - J/128)^T x  (column means removed)
    cmat = np.eye(128, dtype=np.float32) - 1.0 / 128.0

    consts = {
        "wqkT": _bf(wqkT),
        "wvo": _bf(wvo),
        "w1": _bf(w1p),
        "w2r": _bf(w2r),

        "a0t": _bf(A0.T),
        "a1t": _bf(A1.T),
        "gcb3": _bf(gcb3),
        "brows": _bf(brows),
        "bm_p": _bf(_bmask(10)),
        "bm_s": _bf(_bmask(5)),
        "g012": _bf(g012),
        "cmat": _bf(cmat),
        "bvo": _r(bvo[:, None]),
        "fb1r": _r(fb1r),
        "fb2row": _bf(fb2row),
        "lng1c": _r(ln1_g[:, None]),
    }
    return consts


class _Balance:
    """Static per-engine load balancer for flexible elementwise ops."""

    def __init__(self, nc):
        self.nc = nc
        self.load = {"act": 0.0, "dve": 0.0, "pool": 0.0}

    def fixed(self, eng, ns):
        self.load[eng] += ns

    def _pick(self, costs):
        pick = min(costs, key=lambda k: self.load[k] + costs[k])
        self.load[pick] += costs[pick]
        return pick

    def copy(self, dst, src, free, kinds=("act", "dve", "pool")):
        costs = {"act": free * 0.833 + 185.0,
                 "dve": free * 1.042 + 125.0,
                 "pool": free * 1.39 + 131.0}
        pick = self._pick({k: costs[k] for k in kinds})
        if pick == "act":
            self.nc.scalar.copy(dst, src)
        elif pick == "dve":
            self.nc.vector.tensor_copy(dst, src)
        else:
            self.nc.gpsimd.tensor_copy(dst, src)

    def sts(self, out, in0, scalar, in1, op0, op1, free,
            kinds=("dve", "pool")):
        costs = {"dve": free * 1.042 + 125.0,
                 "pool": free * 1.98 + 131.0}
        pick = self._pick({k: costs[k] for k in kinds})
        eng = self.nc.vector if pick == "dve" else self.nc.gpsimd
        eng.scalar_tensor_tensor(out=out, in0=in0, scalar=scalar, in1=in1,
                                 op0=op0, op1=op1)

    def relu(self, out, in_, bias_col, free):
        costs = {"act": free * 0.833 + 160.0,
                 "dve": free * 1.042 + 125.0,
                 "pool": free * 1.98 + 131.0}
        pick = self._pick(costs)
        if pick == "act":
            self.nc.scalar.activation(out, in_, AF.Relu, bias=bias_col,
                                      scale=1.0)
        else:
            eng = self.nc.vector if pick == "dve" else self.nc.gpsimd
            eng.tensor_scalar(out=out, in0=in_, scalar1=bias_col,
                              scalar2=0.0, op0=AL.add, op1=AL.max)


def build_module(bs):
    """Emit the Bass/Tile program for one core handling `bs` batches."""
    TN = T * N
    TT = bs * TN
    nc = bacc.Bacc("TRN2")

    x_d = nc.dram_tensor("x", [bs * N, T, D], F32, kind="ExternalInput")
    out_d = nc.dram_tensor("out", [bs * N, T, D], F32, kind="ExternalOutput")

    cshapes = {
        "wqkT": ([D, H * D], BF), "wvo": ([D, H * D], BF),
        "w1": ([D, DFF], BF), "w2r": ([128, 4, D], BF),
        "a0t": ([N, N], BF), "a1t": ([N, N], BF),
        "gcb3": ([3, 4 * D], BF), "brows": ([3, N], BF),
        "bm_p": ([120, 960], BF), "bm_s": ([120, 480], BF),
        "g012": ([D, 3 * D], BF), "cmat": ([128, 128], BF),
        "bvo": ([D, 1], F32), "fb1r": ([128, 4], F32),
        "fb2row": ([1, D], BF), "lng1c": ([D, 1], F32),
    }
    cd = {k: nc.dram_tensor(k, shp, dt, kind="ExternalInput")
          for k, (shp, dt) in cshapes.items()}

    # per-batch chunks (LN/FFN pipeline granularity)
    bchunks = []
    off = 0
    while off < TN:
        cw = min(512, TN - off)
        bchunks.append((off, cw))
        off += cw

    bal = _Balance(nc)

    with tile.TileContext(nc) as tc, ExitStack() as stk:
        nc_ = nc
        singles = stk.enter_context(tc.tile_pool(name="singles", bufs=1))
        big1 = stk.enter_context(tc.tile_pool(name="big1", bufs=1))
        bigbf = stk.enter_context(tc.tile_pool(name="bigbf", bufs=1))

        # ---- consts to SBUF ----
        csb = {}
        for k, (shp, dt) in cshapes.items():
            if k in ("a0t", "a1t"):
                continue
            t_ = singles.tile(shp, dt, tag=f"c_{k}")
            nc_.sync.dma_start(out=t_, in_=cd[k][...])
            csb[k] = t_
        a_sb = {}
        for k in ("a0t", "a1t"):
            tiles = []
            for mi, (moff, mcnt) in enumerate(NODE_TILES):
                t_ = singles.tile([128, N], BF, tag=f"c_{k}_{mi}")
                nc_.sync.dma_start(out=t_[0:mcnt, :], in_=cd[k][moff:moff + mcnt, :])
                tiles.append(t_)
            a_sb[k] = tiles
        zero_col = singles.tile([128, 1], F32, tag="zero_col")
        nc_.vector.memset(zero_col, 0.0)
        eps_col = singles.tile([128, 1], F32, tag="eps_col")
        nc_.vector.memset(eps_col, LN_EPS)
        ones512_r = singles.tile([1, 512], BF, tag="ones512_r")
        nc_.vector.memset(ones512_r, 1.0)
        ones128_b = singles.tile([128, 128], BF, tag="ones128_b")
        nc_.vector.memset(ones128_b, 1.0)

        # persistent activations: XA holds X1 then (per chunk) X2; XB holds
        # the LN1 output then (per chunk, after FFN consumed it) Z.  The
        # region-level reuse is safe because every rewrite happens after the
        # region's last reader in emission order (slice-level deps).
        XA = big1.tile([128, TT], BF, tag="big1")

        # =========== phase 1+2: input, per-group R, attention ===========
        with tc.tile_pool(name="att_sb", bufs=2) as att, \
             tc.tile_pool(name="stage_sb", bufs=3) as stage_p, \
             tc.tile_pool(name="xb_p", bufs=2) as xb_p, \
             tc.tile_pool(name="r_sb", bufs=3) as r_p, \
             tc.tile_pool(name="ps_r", bufs=1, space="PSUM") as ps_r, \
             tc.tile_pool(name="ps_v", bufs=1, space="PSUM") as ps_v, \
             tc.tile_pool(name="ps_s", bufs=1, space="PSUM") as ps_s, \
             tc.tile_pool(name="ps_c", bufs=2, space="PSUM") as ps_c:
            x_flat = x_d[...].rearrange("r t d -> (r t) d")  # rows == (b,n,t)
            for b in range(bs):
                # ---- input: stage token-major fp32, convert to bf16,
                # transpose to feature-major via the DMA XBAR
                xb = xb_p.tile([128, TN + 64], BF, tag="xbf")
                nc_.vector.memset(xb[:, TN:TN + 64], 0.0)
                o = 0
                while o < TN:
                    gw = min(512, TN - o)
                    st4 = stage_p.tile([128, 4, 128], F32, tag="stage")
                    stb = stage_p.tile([128, 4, 128], BF, tag="stageb")
                    nk = (gw + 127) // 128
                    for k in range(nk):
                        cnt = min(128, gw - k * 128)
                        if cnt < 128:
                            nc_.vector.memset(st4[:, k, :], 0.0)
                        nc_.sync.dma_start(
                            out=st4[0:cnt, k, :],
                            in_=x_flat[b * TN + o + k * 128:
                                       b * TN + o + k * 128 + cnt, :])
                    bal.copy(stb[:, 0:nk, :], st4[:, 0:nk, :], nk * 128)
                    for k in range(nk):
                        cnt = min(128, gw - k * 128)
                        cpad = (cnt + 15) // 16 * 16
                        nc_.sync.dma_start_transpose(
                            out=xb[:, o + k * 128:o + k * 128 + cpad],
                            in_=stb[0:cpad, k, :])
                    o += gw

                # ---- attention over node groups
                for (n0, gn) in GROUPS:
                    gt = gn * 12
                    pw = H * gt
                    xb_g = xb[:, n0 * 12:n0 * 12 + gt]
                    x1_g = XA[:, b * TN + n0 * 12:b * TN + n0 * 12 + gt]

                    # R projection for this group (8 heads, bank-aligned
                    # per-head slots so no matmul crosses a PSUM bank)
                    slot = 128 if gn == 10 else 64
                    xb_gs = xb[:, n0 * 12:n0 * 12 + slot]
                    rp = ps_r.tile([128, 1024], F32, tag="rps")
                    for h in range(H):
                        nc_.tensor.matmul(rp[:, h * slot:(h + 1) * slot],
                                          lhsT=csb["wqkT"][:, h * D:(h + 1) * D],
                                          rhs=xb_gs, start=True, stop=True)
                    r_sb = r_p.tile([128, 1024], BF, tag="rsb")
                    half = (H * slot) // 2
                    bal.copy(r_sb[:, 0:half], rp[:, 0:half], half)
                    bal.copy(r_sb[:, half:H * slot], rp[:, half:H * slot], half)

                    # Vt (token-major) for all 8 heads: (gt, 1024)
                    vt_ps = ps_v.tile([120, 1024], F32, tag="vps")
                    nc_.tensor.matmul(vt_ps[0:gt, 0:512], lhsT=xb_g,
                                      rhs=csb["wvo"][:, 0:512],
                                      start=True, stop=True)
                    nc_.tensor.matmul(vt_ps[0:gt, 512:1024], lhsT=xb_g,
                                      rhs=csb["wvo"][:, 512:1024],
                                      start=True, stop=True)
                    vt = att.tile([120, 1024], BF, tag="vt")
                    bal.copy(vt[0:gt, 0:512], vt_ps[0:gt, 0:512], 512)
                    bal.copy(vt[0:gt, 512:1024], vt_ps[0:gt, 512:1024], 512)

                    # scores into bank-aligned per-head 128 slots
                    sp = ps_s.tile([120, 1024], F32, tag="sps")
                    for h in range(H):
                        nc_.tensor.matmul(
                            sp[0:gt, h * 128:h * 128 + gt],
                            lhsT=r_sb[:, h * slot:h * slot + gt],
                            rhs=xb_g, start=True, stop=True,
                            skip_group_check=True)

                    # softmax on the packed (s, h*gt+t) layout:
                    # exp -> 0/1 block mask -> column sums (bf16) -> divide
                    ph = att.tile([120, 960], BF, tag="ph")
                    nc_.scalar.activation(
                        ph[0:gt, 0:pw].rearrange("p (h c) -> p h c", c=gt),
                        sp[0:gt, :].rearrange("p (h c) -> p h c", c=128)
                        [:, :, 0:gt],
                        AF.Exp, bias=zero_col[0:gt], scale=1.0)
                    bal.fixed("act", pw * 0.833 + 160)
                    bm = csb["bm_p"] if gn == 10 else csb["bm_s"]
                    phm = att.tile([120, 960], BF, tag="phm")
                    nc_.vector.tensor_mul(phm[0:gt, 0:pw], ph[0:gt, 0:pw],
                                          bm[0:gt, 0:pw])
                    bal.fixed("dve", pw * 0.52 + 60)
                    sums = att.tile([120, 960], BF, tag="sums")
                    nc_.gpsimd.partition_all_reduce(
                        sums[0:gt, 0:pw], phm[0:gt, 0:pw],
                        channels=gt, reduce_op=bass_isa.ReduceOp.add)
                    bal.fixed("pool", pw * 0.74 + 160)
                    phn = att.tile([120, 960], BF, tag="phn")
                    nc_.vector.tensor_tensor(out=phn[0:gt, 0:pw],
                                             in0=phm[0:gt, 0:pw],
                                             in1=sums[0:gt, 0:pw],
                                             op=AL.divide)
                    bal.fixed("dve", pw * 0.52 + 60)

                    ctx_ps = ps_c.tile([128, 120], F32, tag="cps")
                    for h in range(H):
                        nc_.tensor.matmul(
                            ctx_ps[:, 0:gt],
                            lhsT=vt[0:gt, h * D:(h + 1) * D],
                            rhs=phn[0:gt, h * gt:(h + 1) * gt],
                            start=(h == 0), stop=(h == H - 1))
                    # residual: X1 = x + attn (+ bvo)
                    bal.sts(out=x1_g, in0=ctx_ps[:, 0:gt],
                            scalar=csb["bvo"][:, 0:1],
                            in1=xb_g, op0=AL.add, op1=AL.add, free=gt)

        # ====== phases 3-6: LN1 -> FFN -> LN2 chunk-pipelined per batch,
        # with the previous batch's GCN interleaved into the emission ======
        XB = bigbf.tile([128, TT], BF, tag="bigbf")

        with tc.tile_pool(name="post_sb", bufs=6) as lnp, \
             tc.tile_pool(name="ffn_sb", bufs=3) as ffp, \
             tc.tile_pool(name="gcn_sb", bufs=2) as gcp, \
             tc.tile_pool(name="gcn_stg", bufs=3) as gst, \
             tc.tile_pool(name="ln_ps", bufs=2, space="PSUM") as lps, \
             tc.tile_pool(name="ffn_ps", bufs=1, space="PSUM") as fps, \
             tc.tile_pool(name="ffn_ps2", bufs=1, space="PSUM") as fps2, \
             tc.tile_pool(name="gcn_pps", bufs=1, space="PSUM") as gps_p, \
             tc.tile_pool(name="gcn_mps", bufs=1, space="PSUM") as gps_m:

            def ln_chunk(src, dst, o, cw):
                xc_ps = lps.tile([128, 512], F32, tag="ln_xc")
                nc_.tensor.matmul(xc_ps[:, 0:cw], lhsT=csb["cmat"],
                                  rhs=src[:, o:o + cw], start=True, stop=True)
                xc = lnp.tile([128, 512], BF, tag="ln_xcs")
                bal.copy(xc[:, 0:cw], xc_ps[:, 0:cw], cw)
                sq = lnp.tile([128, 512], BF, tag="ln_sq")
                nc_.vector.tensor_mul(sq[:, 0:cw], xc[:, 0:cw], xc[:, 0:cw])
                bal.fixed("dve", cw * 0.52 + 60)
                sq_ps = lps.tile([128, 512], F32, tag="ln_sqp")
                nc_.tensor.matmul(sq_ps[:, 0:cw], lhsT=ones128_b,
                                  rhs=sq[:, 0:cw], start=True, stop=True)
                sd = lnp.tile([128, 512], BF, tag="ln_sd")
                nc_.scalar.activation(sd[:, 0:cw], sq_ps[:, 0:cw], AF.Sqrt,
                                      bias=eps_col, scale=1.0 / 128.0)
                bal.fixed("act", cw * 0.833 + 160)
                nc_.vector.tensor_tensor(out=dst[:, o:o + cw],
                                         in0=xc[:, 0:cw], in1=sd[:, 0:cw],
                                         op=AL.divide)
                bal.fixed("dve", cw * 0.52 + 60)

            def ffn_chunk(o, cw):
                h1 = ffp.tile([128, 4, 512], BF, tag="h1")
                for mt in range(4):
                    fp = fps.tile([128, 512], F32, tag="fps")
                    nc_.tensor.matmul(fp[:, 0:cw],
                                      lhsT=csb["w1"][:, mt * 128:(mt + 1) * 128],
                                      rhs=XB[:, o:o + cw],
                                      start=True, stop=True)
                    bal.relu(h1[:, mt, 0:cw], fp[:, 0:cw],
                             csb["fb1r"][:, mt:mt + 1], cw)
                hp = fps2.tile([128, 512], F32, tag="h2ps")
                for kt in range(4):
                    nc_.tensor.matmul(hp[:, 0:cw],
                                      lhsT=csb["w2r"][:, kt, :],
                                      rhs=h1[:, kt, 0:cw],
                                      start=(kt == 0), stop=False,
                                      skip_group_check=True)
                # rank-1: + (ln1_b + ffn_b2) broadcast along tokens
                nc_.tensor.matmul(hp[:, 0:cw], lhsT=csb["fb2row"][0:1, :],
                                  rhs=ones512_r[0:1, 0:cw],
                                  start=False, stop=True,
                                  skip_group_check=True)
                bal.sts(out=XA[:, o:o + cw], in0=XB[:, o:o + cw],
                        scalar=csb["lng1c"][:, 0:1], in1=hp[:, 0:cw],
                        op0=AL.mult, op1=AL.add, free=cw)

            def chunk_units(b):
                for (oo, cw) in bchunks:
                    o = b * TN + oo

                    def unit(o=o, cw=cw):
                        ln_chunk(XA, XB, o, cw)   # LN1: X1 -> zhat1
                        ffn_chunk(o, cw)          # FFN: XB -> XA (X2)
                        ln_chunk(XA, XB, o, cw)   # LN2: X2 -> Z
                    yield unit

            def gcn_units(b):
                zb = XB[:, b * TN:(b + 1) * TN].rearrange(
                    "d (n t) -> d n t", n=N)
                pb = gcp.tile([128, 3, 3, T, 128], BF, tag="pb")
                for t in range(T):
                    for nt, (noff, cnt) in enumerate(NODE_TILES):
                        def unit(t=t, nt=nt, noff=noff, cnt=cnt):
                            pp = gps_p.tile([128, 512], F32, tag="gpps")
                            nc_.tensor.matmul(pp[0:cnt, 0:384],
                                              lhsT=zb[:, noff:noff + cnt, t],
                                              rhs=csb["g012"][:, :],
                                              start=True, stop=True)
                            bal.copy(pb[0:cnt, nt, :, t, :],
                                     pp[0:cnt, 0:384]
                                     .rearrange("p (s e) -> p s e", s=3),
                                     384)
                        yield unit
                for ntile, (noff, cnt_n) in enumerate(NODE_TILES):
                    for c in range(3):
                        def unit(b=b, ntile=ntile, noff=noff,
                                 cnt_n=cnt_n, c=c, pb=pb):
                            mx = gps_m.tile([128, 512], F32, tag="gmps")
                            first = True
                            for sup, akey in ((0, "a0t"), (1, "a1t")):
                                for mt, (moff, cnt_m) in enumerate(NODE_TILES):
                                    nc_.tensor.matmul(
                                        mx[0:cnt_n, :],
                                        lhsT=a_sb[akey][mt][0:cnt_m,
                                                            noff:noff + cnt_n],
                                        rhs=pb[0:cnt_m, mt, sup,
                                               4 * c:4 * c + 4, :],
                                        start=first, stop=False,
                                        skip_group_check=True)
                                    first = False
                            nc_.tensor.matmul(
                                mx[0:cnt_n, :],
                                lhsT=csb["brows"][0:3, noff:noff + cnt_n],
                                rhs=csb["gcb3"][0:3, :],
                                start=False, stop=True,
                                skip_group_check=True)
                            stg = gst.tile([128, 512], F32, tag="ostg")
                            bal.sts(out=stg[0:cnt_n, :], in0=mx[0:cnt_n, :],
                                    scalar=1.0,
                                    in1=pb[0:cnt_n, ntile, 2, 4 * c:4 * c + 4, :],
                                    op0=AL.mult, op1=AL.add, free=512)
                            nc_.sync.dma_start(
                                out=out_d[b * N + noff:b * N + noff + cnt_n,
                                          4 * c:4 * c + 4, :],
                                in_=stg[0:cnt_n, 0:512]
                                .rearrange("n (t d) -> n t d", d=128))
                        yield unit

            # round-robin: batch b's chunk stream interleaved with batch
            # b-1's GCN units
            prev = []
            for b in range(bs):
                cs = list(chunk_units(b))
                gi = 0
                for i, cu in enumerate(cs):
                    cu()
                    want = (i + 1) * len(prev) // len(cs)
                    while gi < want:
                        prev[gi]()
                        gi += 1
                while gi < len(prev):
                    prev[gi]()
                    gi += 1
                prev = list(gcn_units(b))
            for u in prev:
                u()

    nc.compile()
    return nc


_CACHE = {}


def _get_module(bs):
    if bs not in _CACHE:
        _CACHE[bs] = build_module(bs)
    return _CACHE[bs]


def kernel(**inputs):
    from concourse.bass_utils import run_bass_kernel_spmd

    x = np.asarray(inputs["x"], np.float32)
    BN = x.shape[0]
    B = BN // N
    bs = B // NCORES
    consts = make_consts(inputs)
    nc = _get_module(bs)

    in_maps = []
    for c in range(NCORES):
        m = dict(consts)
        m["x"] = _r(x[c * bs * N:(c + 1) * bs * N])
        in_maps.append(m)
    res = run_bass_kernel_spmd(nc, in_maps, list(range(NCORES)))
    out = np.concatenate([res.results[c]["out"] for c in range(NCORES)], axis=0)
    return out.astype(np.float32)


# revision 36
# speedup vs baseline: 1.2652x; 1.2652x over previous
"""Trainium2 Bass kernel for nn_GCEncoderLayer_78400333021790.

GC encoder layer: per-node MHA over T=12 steps + FFN (both with residual+LN),
then a 3-support graph convolution over the 325-node sensor graph.

Strategy (data-parallel over batch B=32 -> 4 batches per core, 8 cores):
  - token order per core: (b, n, t); activations kept feature-major
    X^T = (d=128 partitions, tokens free) so every projection is a natural
    PE matmul.  All persistent activations in bf16.
  - MHA algebra folded on CPU:  S^T = (X Wqk^T) X^T with Wqk = Wq Wk^T/sqrt(128)
    (bq=bk=0 per spec), Vt = X (Wv Wo) so the output projection disappears.
  - groups of 10 nodes (120 tokens) per attention step; block-diagonal mask
    realized as a rank-11 matmul pre-loaded into PSUM (exp underflows to 0).
  - softmax normalization: exp (ACT) -> partition_all_reduce to bf16 (GPSIMD)
    -> single bf16 tensor-tensor divide (DVE, 2x mode).
  - LayerNorm in feature-major: bf16 column sums via ones-matmuls; gamma/beta
    folded into the downstream weights (ffn_W1 / gc_kernel / bias rows), so
    LN emits the bare normalized value via one bf16 divide.
  - PSUM->SBUF evictions load-balanced across ACT / DVE / GPSIMD with a
    static cost model (GPSIMD is otherwise idle).
  - GCN: out = Z G0 + A0 (Z G1) + A1 (Z G2) + bias with dense A built on CPU;
    Z stays in (b, n, t) order and the per-t node tiles are read through
    strided APs.
"""

import os
import sys

for _p in ("/opt/trn_rl_repo", "/root/.axon_site/_ro/trn_rl_repo"):
    if os.path.isdir(_p) and _p not in sys.path:
        sys.path.insert(0, _p)

from contextlib import ExitStack

import ml_dtypes
import numpy as np

import concourse.bass as bass
import concourse.bass_isa as bass_isa
import concourse.tile as tile
from concourse import bacc, mybir

N = 325
T = 12
D = 128
H = 8
DFF = 512
NCORES = 8
B_TOT = 32
LN_EPS = 1e-3
SQRT_D = float(np.sqrt(128.0))

BF = mybir.dt.bfloat16
F32 = mybir.dt.float32
AL = mybir.AluOpType
AF = mybir.ActivationFunctionType
bf16 = ml_dtypes.bfloat16

NODE_TILES = [(0, 128), (128, 128), (256, 69)]
GROUPS = [(i * 10, 10) for i in range(32)] + [(320, 5)]
BIG = 173.0  # sqrt(~30000); exp(-BIG^2) == 0 in fp32


def _r(x):
    return np.ascontiguousarray(x)


def _bf(x):
    return _r(np.asarray(x, np.float32).astype(bf16))


def make_consts(inp):
    """CPU-side weight folding. Returns dict of extra dram inputs (shared
    across cores)."""
    Wq = np.asarray(inp["Wq"], np.float32)
    Wk = np.asarray(inp["Wk"], np.float32)
    Wv = np.asarray(inp["Wv"], np.float32)
    Wo = np.asarray(inp["Wo"], np.float32)
    bv = np.asarray(inp["bv"], np.float32)
    bo = np.asarray(inp["bo"], np.float32)
    ln1_g = np.asarray(inp["ln1_g"], np.float32)
    ln1_b = np.asarray(inp["ln1_b"], np.float32)
    ln2_g = np.asarray(inp["ln2_g"], np.float32)
    ln2_b = np.asarray(inp["ln2_b"], np.float32)

    # wqkT[:, h*D:(h+1)*D][d, e] = Wqk_h[e, d],  Wqk_h = Wq_h Wk_h^T / sqrt(D)
    wqkT = np.empty((D, H * D), np.float32)
    wvo = np.empty((D, H * D), np.float32)
    for h in range(H):
        wqk_h = (Wq[:, h, :] @ Wk[:, h, :].T) / SQRT_D  # (D, D)
        wqkT[:, h * D:(h + 1) * D] = wqk_h.T
        wvo[:, h * D:(h + 1) * D] = Wv[:, h, :] @ Wo[h]  # (D, D)
    bvo = (np.einsum("hk,hkd->d", bv, Wo) + bo).astype(np.float32)

    # block-diag 0/1 masks on the packed (s, h*gt+t) softmax layout
    def _bmask(gn):
        gt = gn * 12
        m = np.zeros((120, H * gt), np.float32)
        for h in range(H):
            for blk in range(gn):
                m[blk * 12:(blk + 1) * 12,
                  h * gt + blk * 12:h * gt + (blk + 1) * 12] = 1.0
        return m

    A0 = np.zeros((N, N), np.float32)
    A1 = np.zeros((N, N), np.float32)
    np.add.at(A0, (np.asarray(inp["sup0_rows"]), np.asarray(inp["sup0_cols"])),
              np.asarray(inp["sup0_vals"], np.float32))
    np.add.at(A1, (np.asarray(inp["sup1_rows"]), np.asarray(inp["sup1_cols"])),
              np.asarray(inp["sup1_vals"], np.float32))

    G = np.asarray(inp["gc_kernel"], np.float32)  # (3D, D), rows ordered (d, m)
    G0, G1, G2 = G[0::3], G[1::3], G[2::3]  # each (D, D)

    # LN2 fold: GCN input is zhat*g2 + b2  ->  scale G rows by g2, push the
    # b2 term into per-support constant rows added via the bias matmul.
    g012 = np.concatenate([ln2_g[:, None] * G1, ln2_g[:, None] * G2,
                           ln2_g[:, None] * G0], axis=1)
    r0 = np.asarray(inp["gc_bias"], np.float32) + ln2_b @ G0
    r1 = ln2_b @ G1
    r2 = ln2_b @ G2
    gcb3 = np.stack([np.tile(r0, 4), np.tile(r1, 4), np.tile(r2, 4)])  # (3,512)
    brows = np.stack([np.ones(N, np.float32), A0.sum(1), A1.sum(1)])  # (3, N)

    # LN1 fold: FFN input is zhat*g1 + b1ln -> scale W1 rows, push b1ln@W1
    # into the relu bias; the residual keeps zhat so the g1 scale rides the
    # final scalar_tensor_tensor and (b1ln + ffn_b2) rides a rank-1 matmul.
    W1 = np.asarray(inp["ffn_W1"], np.float32)
    w1p = ln1_g[:, None] * W1
    b1p = np.asarray(inp["ffn_b1"], np.float32) + ln1_b @ W1
    fb1r = b1p.reshape(4, 128).T  # (128, 4)
    fb2row = (np.asarray(inp["ffn_b2"], np.float32) + ln1_b)[None, :]  # (1,128)

    w2 = np.asarray(inp["ffn_W2"], np.float32)  # (DFF, D)
    w2r = w2.reshape(4, 128, D).transpose(1, 0, 2)  # (128, 4, D)

    # centering matrix for LN: xc = (I - J/128)^T x  (column means removed)
    cmat = np.eye(128, dtype=np.float32) - 1.0 / 128.0

    consts = {
        "wqkT": _bf(wqkT),
        "wvo": _bf(wvo),
        "w1": _bf(w1p),
        "w2r": _bf(w2r),

        "a0t": _bf(A0.T),
        "a1t": _bf(A1.T),
        "gcb3": _bf(gcb3),
        "brows": _bf(brows),
        "bm_p": _bf(_bmask(10)),
        "bm_s": _bf(_bmask(5)),
        "g012": _bf(g012),
        "cmat": _bf(cmat),
        "bvo": _r(bvo[:, None]),
        "fb1r": _r(fb1r),
        "fb2row": _bf(fb2row),
        "lng1c": _r(ln1_g[:, None]),
    }
    return consts


class _Balance:
    """Static per-engine load balancer for flexible elementwise ops."""

    def __init__(self, nc):
        self.nc = nc
        self.load = {"act": 0.0, "dve": 0.0, "pool": 0.0}

    def fixed(self, eng, ns):
        self.load[eng] += ns

    def _pick(self, costs):
        pick = min(costs, key=lambda k: self.load[k] + costs[k])
        self.load[pick] += costs[pick]
        return pick

    def copy(self, dst, src, free, kinds=("act", "dve", "pool")):
        costs = {"act": free * 0.833 + 185.0,
                 "dve": free * 1.042 + 125.0,
                 "pool": free * 1.39 + 131.0}
        pick = self._pick({k: costs[k] for k in kinds})
        if pick == "act":
            self.nc.scalar.copy(dst, src)
        elif pick == "dve":
            self.nc.vector.tensor_copy(dst, src)
        else:
            self.nc.gpsimd.tensor_copy(dst, src)

    def sts(self, out, in0, scalar, in1, op0, op1, free,
            kinds=("dve", "pool")):
        costs = {"dve": free * 1.042 + 125.0,
                 "pool": free * 1.98 + 131.0}
        pick = self._pick({k: costs[k] for k in kinds})
        eng = self.nc.vector if pick == "dve" else self.nc.gpsimd
        eng.scalar_tensor_tensor(out=out, in0=in0, scalar=scalar, in1=in1,
                                 op0=op0, op1=op1)

    def relu(self, out, in_, bias_col, free):
        costs = {"act": free * 0.833 + 160.0,
                 "dve": free * 1.042 + 125.0,
                 "pool": free * 1.98 + 131.0}
        pick = self._pick(costs)
        if pick == "act":
            self.nc.scalar.activation(out, in_, AF.Relu, bias=bias_col,
                                      scale=1.0)
        else:
            eng = self.nc.vector if pick == "dve" else self.nc.gpsimd
            eng.tensor_scalar(out=out, in0=in_, scalar1=bias_col,
                              scalar2=0.0, op0=AL.add, op1=AL.max)


def build_module(bs):
    """Emit the Bass/Tile program for one core handling `bs` batches."""
    TN = T * N
    TT = bs * TN
    nc = bacc.Bacc("TRN2")

    x_d = nc.dram_tensor("x", [bs * N, T, D], F32, kind="ExternalInput")
    out_d = nc.dram_tensor("out", [bs * N, T, D], F32, kind="ExternalOutput")

    cshapes = {
        "wqkT": ([D, H * D], BF), "wvo": ([D, H * D], BF),
        "w1": ([D, DFF], BF), "w2r": ([128, 4, D], BF),
        "a0t": ([N, N], BF), "a1t": ([N, N], BF),
        "gcb3": ([3, 4 * D], BF), "brows": ([3, N], BF),
        "bm_p": ([120, 960], BF), "bm_s": ([120, 480], BF),
        "g012": ([D, 3 * D], BF), "cmat": ([128, 128], BF),
        "bvo": ([D, 1], F32), "fb1r": ([128, 4], F32),
        "fb2row": ([1, D], BF), "lng1c": ([D, 1], F32),
    }
    cd = {k: nc.dram_tensor(k, shp, dt, kind="ExternalInput")
          for k, (shp, dt) in cshapes.items()}

    # per-batch chunks (LN/FFN pipeline granularity)
    bchunks = []
    off = 0
    while off < TN:
        cw = min(512, TN - off)
        bchunks.append((off, cw))
        off += cw

    bal = _Balance(nc)

    with tile.TileContext(nc) as tc, ExitStack() as stk:
        nc_ = nc
        singles = stk.enter_context(tc.tile_pool(name="singles", bufs=1))
        big1 = stk.enter_context(tc.tile_pool(name="big1", bufs=1))
        bigbf = stk.enter_context(tc.tile_pool(name="bigbf", bufs=1))

        # ---- consts to SBUF ----
        csb = {}
        for k, (shp, dt) in cshapes.items():
            if k in ("a0t", "a1t"):
                continue
            t_ = singles.tile(shp, dt, tag=f"c_{k}")
            nc_.sync.dma_start(out=t_, in_=cd[k][...])
            csb[k] = t_
        a_sb = {}
        for k in ("a0t", "a1t"):
            tiles = []
            for mi, (moff, mcnt) in enumerate(NODE_TILES):
                t_ = singles.tile([128, N], BF, tag=f"c_{k}_{mi}")
                nc_.sync.dma_start(out=t_[0:mcnt, :], in_=cd[k][moff:moff + mcnt, :])
                tiles.append(t_)
            a_sb[k] = tiles
        zero_col = singles.tile([128, 1], F32, tag="zero_col")
        nc_.vector.memset(zero_col, 0.0)
        eps_col = singles.tile([128, 1], F32, tag="eps_col")
        nc_.vector.memset(eps_col, LN_EPS)
        ones512_r = singles.tile([1, 512], BF, tag="ones512_r")
        nc_.vector.memset(ones512_r, 1.0)
        ones128_b = singles.tile([128, 128], BF, tag="ones128_b")
        nc_.vector.memset(ones128_b, 1.0)

        # persistent activations: XA holds X1 then (per chunk) X2; XB holds
        # the LN1 output then (per chunk, after FFN consumed it) Z.  The
        # region-level reuse is safe because every rewrite happens after the
        # region's last reader in emission order (slice-level deps).
        XA = big1.tile([128, TT], BF, tag="big1")

        # =========== phase 1+2: input, per-group R, attention ===========
        with tc.tile_pool(name="att_sb", bufs=3) as att, \
             tc.tile_pool(name="stage_sb", bufs=3) as stage_p, \
             tc.tile_pool(name="xb_p", bufs=2) as xb_p, \
             tc.tile_pool(name="r_sb", bufs=3) as r_p, \
             tc.tile_pool(name="ps_r", bufs=1, space="PSUM") as ps_r, \
             tc.tile_pool(name="ps_v", bufs=1, space="PSUM") as ps_v, \
             tc.tile_pool(name="ps_s", bufs=1, space="PSUM") as ps_s, \
             tc.tile_pool(name="ps_c", bufs=2, space="PSUM") as ps_c:
            x_flat = x_d[...].rearrange("r t d -> (r t) d")  # rows == (b,n,t)
            for b in range(bs):
                # ---- input: stage token-major fp32, convert to bf16,
                # transpose to feature-major via the DMA XBAR
                xb = xb_p.tile([128, TN + 64], BF, tag="xbf")
                nc_.vector.memset(xb[:, TN:TN + 64], 0.0)
                o = 0
                while o < TN:
                    gw = min(512, TN - o)
                    st4 = stage_p.tile([128, 4, 128], F32, tag="stage")
                    stb = stage_p.tile([128, 4, 128], BF, tag="stageb")
                    nk = (gw + 127) // 128
                    for k in range(nk):
                        cnt = min(128, gw - k * 128)
                        if cnt < 128:
                            nc_.vector.memset(st4[:, k, :], 0.0)
                        nc_.sync.dma_start(
                            out=st4[0:cnt, k, :],
                            in_=x_flat[b * TN + o + k * 128:
                                       b * TN + o + k * 128 + cnt, :])
                    bal.copy(stb[:, 0:nk, :], st4[:, 0:nk, :], nk * 128)
                    for k in range(nk):
                        cnt = min(128, gw - k * 128)
                        cpad = (cnt + 15) // 16 * 16
                        nc_.sync.dma_start_transpose(
                            out=xb[:, o + k * 128:o + k * 128 + cpad],
                            in_=stb[0:cpad, k, :])
                    o += gw

                # ---- attention over node groups
                for (n0, gn) in GROUPS:
                    gt = gn * 12
                    pw = H * gt
                    xb_g = xb[:, n0 * 12:n0 * 12 + gt]
                    x1_g = XA[:, b * TN + n0 * 12:b * TN + n0 * 12 + gt]

                    # R projection for this group (8 heads, bank-aligned
                    # per-head slots so no matmul crosses a PSUM bank)
                    slot = 128 if gn == 10 else 64
                    xb_gs = xb[:, n0 * 12:n0 * 12 + slot]
                    rp = ps_r.tile([128, 1024], F32, tag="rps")
                    for h in range(H):
                        nc_.tensor.matmul(rp[:, h * slot:(h + 1) * slot],
                                          lhsT=csb["wqkT"][:, h * D:(h + 1) * D],
                                          rhs=xb_gs, start=True, stop=True)
                    r_sb = r_p.tile([128, 1024], BF, tag="rsb")
                    half = (H * slot) // 2
                    bal.copy(r_sb[:, 0:half], rp[:, 0:half], half)
                    bal.copy(r_sb[:, half:H * slot], rp[:, half:H * slot], half)

                    # Vt (token-major) for all 8 heads: (gt, 1024)
                    vt_ps = ps_v.tile([120, 1024], F32, tag="vps")
                    nc_.tensor.matmul(vt_ps[0:gt, 0:512], lhsT=xb_g,
                                      rhs=csb["wvo"][:, 0:512],
                                      start=True, stop=True)
                    nc_.tensor.matmul(vt_ps[0:gt, 512:1024], lhsT=xb_g,
                                      rhs=csb["wvo"][:, 512:1024],
                                      start=True, stop=True)
                    vt = att.tile([120, 1024], BF, tag="vt")
                    bal.copy(vt[0:gt, 0:512], vt_ps[0:gt, 0:512], 512)
                    bal.copy(vt[0:gt, 512:1024], vt_ps[0:gt, 512:1024], 512)

                    # scores into bank-aligned per-head 128 slots
                    sp = ps_s.tile([120, 1024], F32, tag="sps")
                    for h in range(H):
                        nc_.tensor.matmul(
                            sp[0:gt, h * 128:h * 128 + gt],
                            lhsT=r_sb[:, h * slot:h * slot + gt],
                            rhs=xb_g, start=True, stop=True,
                            skip_group_check=True)

                    # softmax on the packed (s, h*gt+t) layout:
                    # exp -> 0/1 block mask -> column sums (bf16) -> divide
                    ph = att.tile([120, 960], BF, tag="ph")
                    nc_.scalar.activation(
                        ph[0:gt, 0:pw].rearrange("p (h c) -> p h c", c=gt),
                        sp[0:gt, :].rearrange("p (h c) -> p h c", c=128)
                        [:, :, 0:gt],
                        AF.Exp, bias=zero_col[0:gt], scale=1.0)
                    bal.fixed("act", pw * 0.833 + 160)
                    bm = csb["bm_p"] if gn == 10 else csb["bm_s"]
                    phm = att.tile([120, 960], BF, tag="phm")
                    nc_.vector.tensor_mul(phm[0:gt, 0:pw], ph[0:gt, 0:pw],
                                          bm[0:gt, 0:pw])
                    bal.fixed("dve", pw * 0.52 + 60)
                    sums = att.tile([120, 960], BF, tag="sums")
                    nc_.gpsimd.partition_all_reduce(
                        sums[0:gt, 0:pw], phm[0:gt, 0:pw],
                        channels=gt, reduce_op=bass_isa.ReduceOp.add)
                    bal.fixed("pool", pw * 0.74 + 160)
                    phn = att.tile([120, 960], BF, tag="phn")
                    nc_.vector.tensor_tensor(out=phn[0:gt, 0:pw],
                                             in0=phm[0:gt, 0:pw],
                                             in1=sums[0:gt, 0:pw],
                                             op=AL.divide)
                    bal.fixed("dve", pw * 0.52 + 60)

                    ctx_ps = ps_c.tile([128, 120], F32, tag="cps")
                    for h in range(H):
                        nc_.tensor.matmul(
                            ctx_ps[:, 0:gt],
                            lhsT=vt[0:gt, h * D:(h + 1) * D],
                            rhs=phn[0:gt, h * gt:(h + 1) * gt],
                            start=(h == 0), stop=(h == H - 1))
                    # residual: X1 = x + attn (+ bvo)
                    bal.sts(out=x1_g, in0=ctx_ps[:, 0:gt],
                            scalar=csb["bvo"][:, 0:1],
                            in1=xb_g, op0=AL.add, op1=AL.add, free=gt)

        # ====== phases 3-6: LN1 -> FFN -> LN2 chunk-pipelined per batch,
        # with the previous batch's GCN interleaved into the emission ======
        XB = bigbf.tile([128, TT], BF, tag="bigbf")

        with tc.tile_pool(name="post_sb", bufs=6) as lnp, \
             tc.tile_pool(name="ffn_sb", bufs=3) as ffp, \
             tc.tile_pool(name="gcn_sb", bufs=2) as gcp, \
             tc.tile_pool(name="gcn_stg", bufs=3) as gst, \
             tc.tile_pool(name="ln_ps", bufs=3, space="PSUM") as lps, \
             tc.tile_pool(name="ffn_ps", bufs=3, space="PSUM") as fps, \
             tc.tile_pool(name="gcn_ps", bufs=2, space="PSUM") as gps:

            def ln_chunk(src, dst, o, cw):
                xc_ps = lps.tile([128, 512], F32, tag="lnp")
                nc_.tensor.matmul(xc_ps[:, 0:cw], lhsT=csb["cmat"],
                                  rhs=src[:, o:o + cw], start=True, stop=True)
                xc = lnp.tile([128, 512], BF, tag="ln_xcs")
                bal.copy(xc[:, 0:cw], xc_ps[:, 0:cw], cw)
                sq = lnp.tile([128, 512], BF, tag="ln_sq")
                nc_.vector.tensor_mul(sq[:, 0:cw], xc[:, 0:cw], xc[:, 0:cw])
                bal.fixed("dve", cw * 0.52 + 60)
                sq_ps = lps.tile([128, 512], F32, tag="lnp")
                nc_.tensor.matmul(sq_ps[:, 0:cw], lhsT=ones128_b,
                                  rhs=sq[:, 0:cw], start=True, stop=True)
                sd = lnp.tile([128, 512], BF, tag="ln_sd")
                nc_.scalar.activation(sd[:, 0:cw], sq_ps[:, 0:cw], AF.Sqrt,
                                      bias=eps_col, scale=1.0 / 128.0)
                bal.fixed("act", cw * 0.833 + 160)
                nc_.vector.tensor_tensor(out=dst[:, o:o + cw],
                                         in0=xc[:, 0:cw], in1=sd[:, 0:cw],
                                         op=AL.divide)
                bal.fixed("dve", cw * 0.52 + 60)

            def ffn_chunk(o, cw):
                h1 = ffp.tile([128, 4, 512], BF, tag="h1")
                for mt in range(4):
                    fp = fps.tile([128, 512], F32, tag="ffnp")
                    nc_.tensor.matmul(fp[:, 0:cw],
                                      lhsT=csb["w1"][:, mt * 128:(mt + 1) * 128],
                                      rhs=XB[:, o:o + cw],
                                      start=True, stop=True)
                    bal.relu(h1[:, mt, 0:cw], fp[:, 0:cw],
                             csb["fb1r"][:, mt:mt + 1], cw)
                hp = fps.tile([128, 512], F32, tag="ffnp")
                for kt in range(4):
                    nc_.tensor.matmul(hp[:, 0:cw],
                                      lhsT=csb["w2r"][:, kt, :],
                                      rhs=h1[:, kt, 0:cw],
                                      start=(kt == 0), stop=False,
                                      skip_group_check=True)
                # rank-1: + (ln1_b + ffn_b2) broadcast along tokens
                nc_.tensor.matmul(hp[:, 0:cw], lhsT=csb["fb2row"][0:1, :],
                                  rhs=ones512_r[0:1, 0:cw],
                                  start=False, stop=True,
                                  skip_group_check=True)
                bal.sts(out=XA[:, o:o + cw], in0=XB[:, o:o + cw],
                        scalar=csb["lng1c"][:, 0:1], in1=hp[:, 0:cw],
                        op0=AL.mult, op1=AL.add, free=cw)

            def chunk_units(b):
                # sub-phase streams keep each phase's chunks adjacent in the
                # engine programs (deep chunk pipelining within LN1 / FFN /
                # LN2) while still pipelining phases per batch
                for (oo, cw) in bchunks:
                    o = b * TN + oo
                    yield lambda o=o, cw=cw: ln_chunk(XA, XB, o, cw)  # LN1
                for (oo, cw) in bchunks:
                    o = b * TN + oo
                    yield lambda o=o, cw=cw: ffn_chunk(o, cw)         # FFN
                for (oo, cw) in bchunks:
                    o = b * TN + oo
                    yield lambda o=o, cw=cw: ln_chunk(XA, XB, o, cw)  # LN2

            def gcn_units(b):
                zb = XB[:, b * TN:(b + 1) * TN].rearrange(
                    "d (n t) -> d n t", n=N)
                pb = gcp.tile([128, 3, 3, T, 128], BF, tag="pb")
                for t in range(T):
                    for nt, (noff, cnt) in enumerate(NODE_TILES):
                        def unit(t=t, nt=nt, noff=noff, cnt=cnt):
                            pp = gps.tile([128, 512], F32, tag="gps")
                            nc_.tensor.matmul(pp[0:cnt, 0:384],
                                              lhsT=zb[:, noff:noff + cnt, t],
                                              rhs=csb["g012"][:, :],
                                              start=True, stop=True)
                            bal.copy(pb[0:cnt, nt, :, t, :],
                                     pp[0:cnt, 0:384]
                                     .rearrange("p (s e) -> p s e", s=3),
                                     384)
                        yield unit
                for ntile, (noff, cnt_n) in enumerate(NODE_TILES):
                    for c in range(3):
                        def unit(b=b, ntile=ntile, noff=noff,
                                 cnt_n=cnt_n, c=c, pb=pb):
                            mx = gps.tile([128, 512], F32, tag="gps")
                            first = True
                            for sup, akey in ((0, "a0t"), (1, "a1t")):
                                for mt, (moff, cnt_m) in enumerate(NODE_TILES):
                                    nc_.tensor.matmul(
                                        mx[0:cnt_n, :],
                                        lhsT=a_sb[akey][mt][0:cnt_m,
                                                            noff:noff + cnt_n],
                                        rhs=pb[0:cnt_m, mt, sup,
                                               4 * c:4 * c + 4, :],
                                        start=first, stop=False,
                                        skip_group_check=True)
                                    first = False
                            nc_.tensor.matmul(
                                mx[0:cnt_n, :],
                                lhsT=csb["brows"][0:3, noff:noff + cnt_n],
                                rhs=csb["gcb3"][0:3, :],
                                start=False, stop=True,
                                skip_group_check=True)
                            stg = gst.tile([128, 512], F32, tag="ostg")
                            bal.sts(out=stg[0:cnt_n, :], in0=mx[0:cnt_n, :],
                                    scalar=1.0,
                                    in1=pb[0:cnt_n, ntile, 2, 4 * c:4 * c + 4, :],
                                    op0=AL.mult, op1=AL.add, free=512)
                            nc_.sync.dma_start(
                                out=out_d[b * N + noff:b * N + noff + cnt_n,
                                          4 * c:4 * c + 4, :],
                                in_=stg[0:cnt_n, 0:512]
                                .rearrange("n (t d) -> n t d", d=128))
                        yield unit

            # round-robin: batch b's chunk stream interleaved with batch
            # b-1's GCN units
            prev = []
            for b in range(bs):
                cs = list(chunk_units(b))
                gi = 0
                for i, cu in enumerate(cs):
                    cu()
                    want = (i + 1) * len(prev) // len(cs)
                    while gi < want:
                        prev[gi]()
                        gi += 1
                while gi < len(prev):
                    prev[gi]()
                    gi += 1
                prev = list(gcn_units(b))
            for u in prev:
                u()

    nc.compile()
    return nc


_CACHE = {}


def _get_module(bs):
    if bs not in _CACHE:
        _CACHE[bs] = build_module(bs)
    return _CACHE[bs]


def kernel(**inputs):
    from concourse.bass_utils import run_bass_kernel_spmd

    x = np.asarray(inputs["x"], np.float32)
    BN = x.shape[0]
    B = BN // N
    bs = B // NCORES
    consts = make_consts(inputs)
    nc = _get_module(bs)

    in_maps = []
    for c in range(NCORES):
        m = dict(consts)
        m["x"] = _r(x[c * bs * N:(c + 1) * bs * N])
        in_maps.append(m)
    res = run_bass_kernel_spmd(nc, in_maps, list(range(NCORES)))
    out = np.concatenate([res.results[c]["out"] for c in range(NCORES)], axis=0)
    return out.astype(np.float32)
